# revision 1
# baseline (speedup 1.0000x reference)
"""Trainium2 Bass kernel for nn_SoftBiasTransformer.

3-layer post-norm transformer encoder, B=1024 S=64 D=768 H=6 HD=128 FF=3072,
with a learned [S,S] additive attention bias shared across batch/heads.

Strategy:
- Data-parallel over batch across 8 NeuronCores (128 batches/core).
- fp16 matmul operands (full TensorE rate, ~3e-4 rel precision), fp32 PSUM
  accumulation, fp32 layernorm statistics.
- Feature-major activations [D(part), tokens(free)]: dense chain computes
  outT = W.T @ xT with weights stationary, no activation transposes. V is
  produced token-major (x stationary) so the attention context matmul can
  consume PE-transposed softmax probabilities directly.
- LayerNorm mean/var via (1/D)-vector matmuls on TensorE (reduction over
  the feature/partition axis), broadcast back with K=1 matmuls.
- Softmax bias folded in multiplicatively: exp(s+b) = exp(s)*exp(b), with
  exp(bias) precomputed on the host from sp_table[sp_matrix]. The 1/sqrt(HD)
  score scale is folded into Wq/bq on the host.
"""

import math

import numpy as np

B, S, D = 1024, 64, 768
H, HD, FF, L = 6, 128, 3072, 3
NCORES = 8
BS = B // NCORES            # batches per core = 128
TOK = BS * S                # tokens per core = 8192
KD = D // 128               # 6
KF = FF // 128              # 24
EPS = 1e-5
TC = 1024                   # tokens per chunk
BC = TC // S                # batches per chunk = 16
NQ = TC // 512              # 512-col units per chunk = 2
T8 = TC // 128              # 128-token blocks per chunk = 8

_CACHED_NC = {}


def _build_nc(n_chunks, stage="full"):
    import concourse.tile as tile
    import concourse.mybir as mybir
    from concourse import bacc
    from contextlib import ExitStack

    f16 = mybir.dt.float16
    f32 = mybir.dt.float32
    Alu = mybir.AluOpType
    Act = mybir.ActivationFunctionType

    nc = bacc.Bacc("TRN2", target_bir_lowering=False, debug=False,
                   enable_asserts=False, num_devices=1)

    # ---- DRAM I/O ----
    xw = nc.dram_tensor("xw", [KD, 128, TOK], f16, kind="ExternalInput")
    Wq_s = nc.dram_tensor("Wq_s", [L, KD, 128, D], f16, kind="ExternalInput")
    Wk_s = nc.dram_tensor("Wk_s", [L, KD, 128, D], f16, kind="ExternalInput")
    Wv_s = nc.dram_tensor("Wv_s", [L, KD, 128, D], f16, kind="ExternalInput")
    Wo_s = nc.dram_tensor("Wo_s", [L, KD, 128, D], f16, kind="ExternalInput")
    W1_s = nc.dram_tensor("W1_s", [L, KD, 128, FF], f16, kind="ExternalInput")
    W2_s = nc.dram_tensor("W2_s", [L, KF, 128, D], f16, kind="ExternalInput")
    bq_t = nc.dram_tensor("bq_t", [L, 128, KD], f32, kind="ExternalInput")
    bk_t = nc.dram_tensor("bk_t", [L, 128, KD], f32, kind="ExternalInput")
    bv_t = nc.dram_tensor("bv_t", [L, 128, KD], f32, kind="ExternalInput")
    bo_t = nc.dram_tensor("bo_t", [L, 128, KD], f32, kind="ExternalInput")
    b1_t = nc.dram_tensor("b1_t", [L, 128, KF], f32, kind="ExternalInput")
    b2_t = nc.dram_tensor("b2_t", [L, 128, KD], f32, kind="ExternalInput")
    l1s_t = nc.dram_tensor("l1s_t", [L, 128, KD], f32, kind="ExternalInput")
    l1b_t = nc.dram_tensor("l1b_t", [L, 128, KD], f32, kind="ExternalInput")
    l2s_t = nc.dram_tensor("l2s_t", [L, 128, KD], f32, kind="ExternalInput")
    l2b_t = nc.dram_tensor("l2b_t", [L, 128, KD], f32, kind="ExternalInput")
    eb2 = nc.dram_tensor("eb2", [128, S], f16, kind="ExternalInput")
    id16 = nc.dram_tensor("id16", [128, 128], f16, kind="ExternalInput")
    y = nc.dram_tensor("y", [TOK, D], f32, kind="ExternalOutput")

    inv_d = 1.0 / D

    with tile.TileContext(nc) as tc, ExitStack() as ctx:
        consts = ctx.enter_context(tc.tile_pool(name="consts", bufs=1))
        p_x = ctx.enter_context(tc.tile_pool(name="p_x", bufs=1))
        p_act = ctx.enter_context(tc.tile_pool(name="p_act", bufs=1))
        p_sm = ctx.enter_context(tc.tile_pool(name="p_sm", bufs=2))
        p_sq = ctx.enter_context(tc.tile_pool(name="p_sq", bufs=2))
        p_h = ctx.enter_context(tc.tile_pool(name="p_h", bufs=1))
        p_row = ctx.enter_context(tc.tile_pool(name="p_row", bufs=2))
        p_out = ctx.enter_context(tc.tile_pool(name="p_out", bufs=2))
        p_w = ctx.enter_context(tc.tile_pool(name="p_w", bufs=2))
        p_w1 = ctx.enter_context(tc.tile_pool(name="p_w1", bufs=1))
        p_w2 = ctx.enter_context(tc.tile_pool(name="p_w2", bufs=1))
        ps_mm = ctx.enter_context(tc.tile_pool(name="ps_mm", bufs=4, space="PSUM"))
        ps_st = ctx.enter_context(tc.tile_pool(name="ps_st", bufs=2, space="PSUM"))
        ps_bc = ctx.enter_context(tc.tile_pool(name="ps_bc", bufs=2, space="PSUM"))

        # ---- constants ----
        ones_col = consts.tile([128, 1], f16)       # value 1/D: LN sum lhsT
        nc.vector.memset(ones_col, inv_d)
        ones_row = consts.tile([1, 128], f16)       # K=1 broadcast lhsT
        nc.vector.memset(ones_row, 1.0)
        eps_t = consts.tile([1, 1], f32)
        nc.vector.memset(eps_t, EPS)
        eb2_sb = consts.tile([128, S], f16)
        nc.sync.dma_start(eb2_sb[:], eb2.ap())
        id16_sb = consts.tile([128, 128], f16)
        nc.sync.dma_start(id16_sb[:], id16.ap())

        per_layer = {}
        for l in range(L):
            d = {}
            for name, dram, w in [
                ("bq", bq_t, KD), ("bk", bk_t, KD), ("bv", bv_t, KD),
                ("bo", bo_t, KD), ("b1", b1_t, KF), ("b2", b2_t, KD),
                ("l1s", l1s_t, KD), ("l1b", l1b_t, KD),
                ("l2s", l2s_t, KD), ("l2b", l2b_t, KD),
            ]:
                t = consts.tile([128, w], f32, tag=f"{name}_{l}")
                nc.sync.dma_start(t[:], dram.ap()[l])
                d[name] = t
            per_layer[l] = d

        def load_w(dram, l, nk, width, pool, tag):
            t = pool.tile([128, nk, width], f16, tag=tag)
            nc.sync.dma_start(t[:], dram.ap()[l].rearrange("k p f -> p k f"))
            return t

        def layer_norm(s_in, gamma, beta, out_sb):
            """Feature-major LN over D: out = (s-mu)*rstd*gamma+beta (fp16)."""
            for half in range(NQ):
                sl = slice(half * 512, half * 512 + 512)
                mu_ps = ps_st.tile([1, 512], f32, tag="st")
                msq_ps = ps_st.tile([1, 512], f32, tag="st")
                for k in range(KD):
                    nc.tensor.matmul(mu_ps[:], ones_col[:], s_in[:, k, sl],
                                     start=(k == 0), stop=(k == KD - 1))
                for k in range(KD):
                    sq = p_sq.tile([128, 512], f16, tag="sq")
                    nc.scalar.activation(sq[:], s_in[:, k, sl], Act.Square)
                    nc.tensor.matmul(msq_ps[:], ones_col[:], sq[:],
                                     start=(k == 0), stop=(k == KD - 1))
                # var = msq - mu^2 ; rstd = 1/sqrt(var+eps)
                musq = p_row.tile([1, 512], f32, tag="row32")
                nc.scalar.activation(musq[:], mu_ps[:], Act.Square)
                var = p_row.tile([1, 512], f32, tag="row32")
                nc.vector.tensor_tensor(var[:], msq_ps[:], musq[:], Alu.subtract)
                std = p_row.tile([1, 512], f32, tag="row32")
                nc.scalar.activation(std[:], var[:], Act.Sqrt, bias=eps_t[:])
                rstd = p_row.tile([1, 512], f16, tag="rstd")
                with nc.allow_low_precision(reason="fp16 rstd for broadcast mm"):
                    nc.vector.reciprocal(rstd[:], std[:])
                mu16 = p_row.tile([1, 512], f16, tag="mu16")
                nc.scalar.activation(mu16[:], mu_ps[:], Act.Copy)
                # broadcast over partitions via K=1 matmuls
                mu_b = ps_bc.tile([128, 512], f32, tag="bc")
                nc.tensor.matmul(mu_b[:], ones_row[:], mu16[:],
                                 start=True, stop=True)
                rstd_b = ps_bc.tile([128, 512], f32, tag="bc")
                nc.tensor.matmul(rstd_b[:], ones_row[:], rstd[:],
                                 start=True, stop=True)
                for m in range(KD):
                    t0 = p_sq.tile([128, 512], f16, tag="lnt")
                    nc.vector.scalar_tensor_tensor(
                        t0[:], s_in[:, m, sl], 1.0, mu_b[:],
                        Alu.mult, Alu.subtract)
                    nc.vector.scalar_tensor_tensor(
                        t0[:], t0[:], gamma[:, m:m + 1], rstd_b[:],
                        Alu.mult, Alu.mult)
                    nc.vector.tensor_scalar_add(
                        out_sb[:, m, sl], t0[:], beta[:, m:m + 1])

        def dump(tile_f16, c):
            """Debug: cast a [128,*] f16 tile to f32 and DMA into y (flat)."""
            yf = y.ap().rearrange("t d -> (t d)").rearrange(
                "(p f) -> p f", p=128)
            flat = tile_f16[:]
            if len(flat.shape) == 3:
                flat = flat.rearrange("p a b -> p (a b)")
            np_, n = flat.shape
            for q in range(n // 512):
                t32 = p_out.tile([128, 512], f32, tag="dump")
                nc.scalar.activation(t32[:np_], flat[:, q * 512:(q + 1) * 512],
                                     Act.Copy)
                nc.sync.dma_start(
                    yf[:np_, c * n + q * 512: c * n + (q + 1) * 512],
                    t32[:np_])

        # ---------------- main program ----------------
        for c in range(n_chunks):
            tok0 = c * TC
            x16 = p_x.tile([128, KD, TC], f16, tag="x16")
            nc.sync.dma_start(
                x16[:], xw.ap()[:, :, tok0:tok0 + TC].rearrange("o p t -> p o t"))

            for l in range(L):
                cl = per_layer[l]
                xin = x16

                # --- Q, K projections (feature-major) ---
                wq = load_w(Wq_s, l, KD, D, p_w, "wqkvo")
                q16 = p_act.tile([128, KD, TC], f16, tag="q16")
                wk = load_w(Wk_s, l, KD, D, p_w, "wqkvo")
                k16 = p_act.tile([128, KD, TC], f16, tag="k16")
                for w_sl, out_sb, bias in ((wq, q16, cl["bq"]),
                                           (wk, k16, cl["bk"])):
                    for m in range(KD):
                        for q in range(NQ):
                            ps = ps_mm.tile([128, 512], f32, tag="mm")
                            for k in range(KD):
                                nc.tensor.matmul(
                                    ps[:],
                                    w_sl[:, k, m * 128:(m + 1) * 128],
                                    xin[:, k, q * 512:(q + 1) * 512],
                                    start=(k == 0), stop=(k == KD - 1))
                            nc.scalar.activation(
                                out_sb[:, m, q * 512:(q + 1) * 512], ps[:],
                                Act.Identity, bias=bias[:, m:m + 1], scale=1.0)

                if stage == "qk":
                    dump(q16, c)
                    break

                # --- V token-major per batch: v[s, batch, hd] (base 0) ---
                wv = load_w(Wv_s, l, KD, D, p_w, "wqkvo")
                v16a = p_act.tile([64, BC // 2, D], f16, tag="v16")
                v16b = p_h.tile([64, BC // 2, D], f16, tag="h16")

                def vslice(b, cols):
                    t = v16a if b < BC // 2 else v16b
                    return t[:, b % (BC // 2), cols]

                for b in range(BC):
                    for nh in range(2):
                        ps = ps_mm.tile([64, 384], f32, tag="mm")
                        for k in range(KD):
                            nc.tensor.matmul(
                                ps[:],
                                xin[:, k, b * 64:(b + 1) * 64],
                                wv[:, k, nh * 384:(nh + 1) * 384],
                                start=(k == 0), stop=(k == KD - 1))
                        nc.scalar.activation(
                            vslice(b, slice(nh * 384, (nh + 1) * 384)),
                            ps[:], Act.Copy)

                if stage == "v":
                    dump(v16a, c)
                    break

                # --- attention per head ---
                wo = load_w(Wo_s, l, KD, D, p_w, "wqkvo")
                ctx16 = p_act.tile([128, KD, TC], f16, tag="ctx16")
                for h in range(H):
                    # scores: 16 batches in one [128,512] psum tile
                    sc_ps = ps_mm.tile([128, 512], f32, tag="mm")
                    for b in range(BC):
                        p_slot = b % 2
                        j = b // 2
                        nc.tensor.matmul(
                            sc_ps[64 * p_slot:64 * p_slot + 64,
                                  j * 64:(j + 1) * 64],
                            q16[:, h, b * 64:(b + 1) * 64],
                            k16[:, h, b * 64:(b + 1) * 64],
                            start=True, stop=True,
                            tile_position=(0, 64 * p_slot))
                    # softmax over keys: p = exp(s)*exp(bias) / sum
                    ex = p_sm.tile([128, 8, S], f16, tag="ex")
                    nc.scalar.activation(
                        ex[:].rearrange("p a b -> p (a b)"), sc_ps[:], Act.Exp)
                    if stage == "attn_sc":
                        dump(ex, c)
                        break
                    nc.vector.tensor_tensor(
                        ex[:], ex[:],
                        eb2_sb[:, None, :].to_broadcast((128, 8, S)),
                        Alu.mult)
                    sums = p_row.tile([128, 8], f32, tag="sums")
                    nc.vector.reduce_sum(sums[:], ex[:],
                                         axis=mybir.AxisListType.X)
                    rec = p_row.tile([128, 8], f32, tag="rec")
                    nc.vector.reciprocal(rec[:], sums[:])
                    for j in range(8):
                        nc.vector.tensor_scalar_mul(
                            ex[:, j, :], ex[:, j, :], rec[:, j:j + 1])
                    if stage == "attn_sm":
                        dump(ex, c)
                        break
                    # transpose probs on PE -> pT (keys at partitions 0:63)
                    pTs = []
                    for halfj in range(2):
                        tp_ps = ps_mm.tile([128, 512], f16, tag="mm")
                        for jj in range(4):
                            j = halfj * 4 + jj
                            nc.tensor.transpose(
                                tp_ps[:64, jj * 128:(jj + 1) * 128],
                                ex[:, j, :], id16_sb[:])
                        pT = p_sm.tile([64, 512], f16, tag="pT")
                        nc.scalar.activation(pT[:], tp_ps[:64, :], Act.Copy)
                        pTs.append(pT)
                    if stage == "attn_tp":
                        dump(pTs[0], c)
                        break
                    # context: ctxT[hd, q] = v.T @ probsT, per batch
                    for half in range(NQ):
                        cx_ps = ps_mm.tile([128, 512], f32, tag="mm")
                        for bb in range(8):
                            b = half * 8 + bb
                            p_slot = b % 2
                            j = b // 2
                            pT = pTs[j // 4]
                            nc.tensor.matmul(
                                cx_ps[:, bb * 64:(bb + 1) * 64],
                                vslice(b, slice(h * 128, (h + 1) * 128)),
                                pT[:, (j % 4) * 128 + 64 * p_slot:
                                   (j % 4) * 128 + 64 * p_slot + 64],
                                start=True, stop=True)
                        nc.scalar.activation(
                            ctx16[:, h, half * 512:(half + 1) * 512],
                            cx_ps[:], Act.Identity,
                            bias=cl["bv"][:, h:h + 1], scale=1.0)

                if stage in ("attn_sc", "attn_sm", "attn_tp"):
                    break
                if stage == "attn":
                    dump(ctx16, c)
                    break

                # --- Wo + residual -> s1, then LN1 -> z16 ---
                s1 = p_act.tile([128, KD, TC], f16, tag="q16")
                for m in range(KD):
                    for q in range(NQ):
                        ps = ps_mm.tile([128, 512], f32, tag="mm")
                        for k in range(KD):
                            nc.tensor.matmul(
                                ps[:],
                                wo[:, k, m * 128:(m + 1) * 128],
                                ctx16[:, k, q * 512:(q + 1) * 512],
                                start=(k == 0), stop=(k == KD - 1))
                        nc.vector.scalar_tensor_tensor(
                            s1[:, m, q * 512:(q + 1) * 512], ps[:],
                            cl["bo"][:, m:m + 1],
                            xin[:, m, q * 512:(q + 1) * 512],
                            Alu.add, Alu.add)
                z16 = p_act.tile([128, KD, TC], f16, tag="k16")
                layer_norm(s1, cl["l1s"], cl["l1b"], z16)
                if stage == "ln1":
                    dump(z16, c)
                    break

                # --- FFN ---
                w1 = load_w(W1_s, l, KD, FF, p_w1, "w1")
                w2 = load_w(W2_s, l, KF, D, p_w2, "w2")
                last = (l == L - 1)
                if not last:
                    xout = p_x.tile([128, KD, TC], f16, tag="x16")
                s2 = p_act.tile([128, KD, TC], f16, tag="v16")
                for q in range(NQ):
                    qsl = slice(q * 512, q * 512 + 512)
                    h16 = p_h.tile([128, KF, 512], f16, tag="h16")
                    for m in range(KF):
                        ps = ps_mm.tile([128, 512], f32, tag="mm")
                        for k in range(KD):
                            nc.tensor.matmul(
                                ps[:],
                                w1[:, k, m * 128:(m + 1) * 128],
                                z16[:, k, qsl],
                                start=(k == 0), stop=(k == KD - 1))
                        nc.scalar.activation(
                            h16[:, m, :], ps[:], Act.Relu,
                            bias=cl["b1"][:, m:m + 1], scale=1.0)
                    for m in range(KD):
                        ps = ps_mm.tile([128, 512], f32, tag="mm")
                        for k in range(KF):
                            nc.tensor.matmul(
                                ps[:],
                                w2[:, k, m * 128:(m + 1) * 128],
                                h16[:, k, :],
                                start=(k == 0), stop=(k == KF - 1))
                        nc.vector.scalar_tensor_tensor(
                            s2[:, m, qsl], ps[:], cl["b2"][:, m:m + 1],
                            z16[:, m, qsl], Alu.add, Alu.add)

                if stage == "ffn":
                    dump(s2, c)
                    break

                # --- LN2 ---
                if last:
                    x2 = p_act.tile([128, KD, TC], f16, tag="ctx16")
                    layer_norm(s2, cl["l2s"], cl["l2b"], x2)
                    # transpose to token-major fp32 and store
                    for t in range(T8):
                        ps_a = ps_mm.tile([128, 512], f16, tag="mm")
                        ps_b = ps_mm.tile([128, 512], f16, tag="mm")
                        for po in range(KD):
                            tgt = ps_a if po < 4 else ps_b
                            off = (po % 4) * 128
                            nc.tensor.transpose(
                                tgt[:, off:off + 128],
                                x2[:, po, t * 128:(t + 1) * 128],
                                id16_sb[:])
                        ob = p_out.tile([128, KD, 128], f32, tag="ob")
                        nc.scalar.activation(
                            ob[:, :4, :].rearrange("p a b -> p (a b)"),
                            ps_a[:], Act.Copy)
                        nc.scalar.activation(
                            ob[:, 4:, :].rearrange("p a b -> p (a b)"),
                            ps_b[:, :256], Act.Copy)
                        nc.sync.dma_start(
                            y.ap()[tok0 + t * 128: tok0 + (t + 1) * 128, :],
                            ob[:].rearrange("p a b -> p (a b)"))
                else:
                    layer_norm(s2, cl["l2s"], cl["l2b"], xout)
                    x16 = xout

    nc.finalize()
    return nc


def _host_prep(inputs):
    x = np.asarray(inputs["x"])
    scale = 1.0 / math.sqrt(HD)
    f16 = np.float16
    f32 = np.float32

    def slabs(w, nk):
        return np.ascontiguousarray(
            np.asarray(w).reshape(L, nk, 128, np.asarray(w).shape[-1])
        ).astype(f16)

    def cols(b, nk):  # [L, feat] -> [L, 128, nk]
        return np.ascontiguousarray(
            np.asarray(b, f32).reshape(L, nk, 128).transpose(0, 2, 1))

    prep = {
        "Wq_s": (np.asarray(inputs["Wq"]) * scale)
        .reshape(L, KD, 128, D).astype(f16),
        "Wk_s": slabs(inputs["Wk"], KD),
        "Wv_s": slabs(inputs["Wv"], KD),
        "Wo_s": slabs(inputs["Wo"], KD),
        "W1_s": slabs(inputs["W1"], KD),
        "W2_s": slabs(inputs["W2"], KF),
        "bq_t": cols(np.asarray(inputs["bq"]) * scale, KD),
        "bk_t": cols(inputs["bk"], KD),
        "bv_t": cols(inputs["bv"], KD),
        "bo_t": cols(inputs["bo"], KD),
        "b1_t": cols(inputs["b1"], KF),
        "b2_t": cols(inputs["b2"], KD),
        "l1s_t": cols(inputs["ln1_s"], KD),
        "l1b_t": cols(inputs["ln1_b"], KD),
        "l2s_t": cols(inputs["ln2_s"], KD),
        "l2b_t": cols(inputs["ln2_b"], KD),
    }
    prep = {k: np.ascontiguousarray(v) for k, v in prep.items()}

    bias = np.asarray(inputs["sp_table"])[np.asarray(inputs["sp_matrix"])]
    eb = np.exp(bias.astype(np.float64)).astype(f16)
    prep["eb2"] = np.ascontiguousarray(np.concatenate([eb, eb], axis=0))
    prep["id16"] = np.eye(128, dtype=f16)

    # x: [B, S, D] -> per-core feature-major fp16 [NCORES, KD, 128, TOK]
    x16 = x.astype(f16).reshape(NCORES, TOK, KD, 128)
    xw = np.ascontiguousarray(x16.transpose(0, 2, 3, 1))
    return prep, xw


def kernel(**inputs) -> np.ndarray:
    from concourse import bass_utils

    n_chunks = int(inputs.pop("_n_chunks", TOK // TC))
    trace = bool(inputs.pop("_trace", False))

    if n_chunks not in _CACHED_NC:
        _CACHED_NC[n_chunks] = _build_nc(n_chunks)
    nc = _CACHED_NC[n_chunks]

    prep, xw = _host_prep(inputs)
    in_maps = [dict(prep, xw=np.ascontiguousarray(xw[c]))
               for c in range(NCORES)]

    res = bass_utils.run_bass_kernel_spmd(
        nc, in_maps, core_ids=list(range(NCORES)), trace=trace)
    kernel.last_result = res

    out = np.zeros((B, S, D), dtype=np.float32)
    ntok = n_chunks * TC
    for c in range(NCORES):
        yc = res.results[c]["y"][:ntok]
        out[c * BS: c * BS + ntok // S] = yc.reshape(ntok // S, S, D)
    return out



# revision 20
# speedup vs baseline: 1.1860x; 1.1860x over previous
"""Trainium2 Bass kernel for nn_SoftBiasTransformer (v2).

3-layer post-norm transformer encoder, B=1024 S=64 D=768 H=6 HD=128 FF=3072,
learned [S,S] additive attention bias shared across batch/heads.

Strategy (v2):
- Data-parallel over batch across 8 NeuronCores (128 batches/core).
- Per core: 4 passes x 32 batches; per pass: layer-outer, phase-split
  (attention sweep over 4 chunks of 512 tokens, then FFN sweep), with a
  software pipeline: chunk c's dependency tail (softmax normalize, LN
  chains) is emitted interleaved with chunk c+1's dense projections so
  TensorE never idles (keeps the HAM clock gate at 8/8).
- Transposed scores: scT[k,q] = k16^T q16 per batch (col-tiled pairs), the
  [S,S] log-bias added in PSUM via an identity matmul, softmax denominators
  via a slot-selector matmul, reciprocal via Ln/Exp on ScalarE (stays in
  the natural_log_exp table set -> no ACT table switches), normalization
  broadcast back with a K=2 matmul. No PE transposes anywhere.
- V computed token-major with 2 batches packed per 128 partitions (full
  M=128 efficiency); context matmuls per batch with row slots.
- LayerNorm gamma/beta folded into downstream weights on the host; device
  LNs produce plain (s-mu)*rstd. mean/meansq via (1/D)-matmuls into one
  PSUM tile (rows 0/32), rstd = exp(-0.5*ln(var+eps)).
- fp16 operands, fp32 PSUM, feature-major activations [D(part), tok].
- Output written feature-major fp32; host transposes to [B, S, D].
"""

import math

import numpy as np

B, S, D = 1024, 64, 768
H, HD, FF, L = 6, 128, 3072, 3
NCORES = 8
BS = B // NCORES            # batches per core = 128
TOK = BS * S                # tokens per core = 8192
KD = D // 128               # 6
KF = FF // 128              # 24
KF2 = KF // 2               # 12
EPS = 1e-5
TC = 512                    # tokens per chunk
BC = TC // S                # batches per chunk = 8
NPASS = 8                   # passes per core
NCH = 2                     # chunks per pass
SC_W = (BC // 2) * S        # scores psum width = 256

_CACHED_NC = {}


def _build_nc(npass=NPASS, nchunk=NCH, dbg=9):
    import concourse.tile as tile
    import concourse.mybir as mybir
    from concourse import bacc
    from contextlib import ExitStack

    f16 = mybir.dt.float16
    f32 = mybir.dt.float32
    Alu = mybir.AluOpType
    Act = mybir.ActivationFunctionType

    nc = bacc.Bacc("TRN2", target_bir_lowering=False, debug=False,
                   enable_asserts=False, num_devices=1)

    ntok = npass * nchunk * TC

    # ---- DRAM I/O ----
    xw = nc.dram_tensor("xw", [KD, 128, TOK], f16, kind="ExternalInput")
    Wq_s = nc.dram_tensor("Wq_s", [L, KD, 128, D], f16, kind="ExternalInput")
    Wk_s = nc.dram_tensor("Wk_s", [L, KD, 128, D], f16, kind="ExternalInput")
    Wv_s = nc.dram_tensor("Wv_s", [L, KD, 128, D], f16, kind="ExternalInput")
    Wo_s = nc.dram_tensor("Wo_s", [L, KD, 128, D], f16, kind="ExternalInput")
    W1_s = nc.dram_tensor("W1_s", [L, KD, 128, FF], f16, kind="ExternalInput")
    W2_s = nc.dram_tensor("W2_s", [L, KF, 128, D], f16, kind="ExternalInput")
    bq_t = nc.dram_tensor("bq_t", [L, 128, KD], f32, kind="ExternalInput")
    bk_t = nc.dram_tensor("bk_t", [L, 128, KD], f32, kind="ExternalInput")
    b1_t = nc.dram_tensor("b1_t", [L, 128, KF], f32, kind="ExternalInput")
    c1_t = nc.dram_tensor("c1_t", [L, 128, KD], f32, kind="ExternalInput")
    g1_t = nc.dram_tensor("g1_t", [L, 128, KD], f32, kind="ExternalInput")
    c2_t = nc.dram_tensor("c2_t", [L, 128, KD], f32, kind="ExternalInput")
    g2_t = nc.dram_tensor("g2_t", [L, 128, KD], f32, kind="ExternalInput")
    go_t = nc.dram_tensor("go_t", [128, KD], f32, kind="ExternalInput")
    boc_t = nc.dram_tensor("boc_t", [128, KD], f32, kind="ExternalInput")
    logbT4 = nc.dram_tensor("logbT4", [64, SC_W], f16, kind="ExternalInput")
    selT_d = nc.dram_tensor("selT_d", [2, 128], f16, kind="ExternalInput")
    id64_d = nc.dram_tensor("id64_d", [64, 64], f16, kind="ExternalInput")
    y = nc.dram_tensor("y", [128, KD, ntok], f32, kind="ExternalOutput")

    import os
    inv_d = 1.0 / D
    pipe_mode = int(os.environ.get("PIPE_MODE", "3"))
    pipelined = nchunk >= 2

    with tile.TileContext(nc) as tc, ExitStack() as ctx:
        consts = ctx.enter_context(tc.tile_pool(name="consts", bufs=1))
        p_x = ctx.enter_context(tc.tile_pool(name="p_x", bufs=1))
        p_w = ctx.enter_context(tc.tile_pool(name="p_w", bufs=3))
        p_w1 = ctx.enter_context(tc.tile_pool(name="p_w1", bufs=1))
        p_w2 = ctx.enter_context(tc.tile_pool(name="p_w2", bufs=1))
        p_q = ctx.enter_context(tc.tile_pool(name="p_q", bufs=1))
        p_k = ctx.enter_context(tc.tile_pool(name="p_k", bufs=1))
        p_v = ctx.enter_context(tc.tile_pool(name="p_v", bufs=1))
        p_vlo = ctx.enter_context(tc.tile_pool(name="p_vlo", bufs=1))
        p_plo = ctx.enter_context(tc.tile_pool(name="p_plo", bufs=3))
        p_ctx = ctx.enter_context(tc.tile_pool(name="p_ctx", bufs=1))
        p_s = ctx.enter_context(tc.tile_pool(name="p_s", bufs=1))
        p_h = ctx.enter_context(tc.tile_pool(name="p_h", bufs=1))
        p_ex = ctx.enter_context(tc.tile_pool(name="p_ex", bufs=6))
        p_p = ctx.enter_context(tc.tile_pool(name="p_p", bufs=3))
        p_t = ctx.enter_context(tc.tile_pool(name="p_t", bufs=3))
        p_sq = ctx.enter_context(tc.tile_pool(name="p_sq", bufs=2))
        p_row = ctx.enter_context(tc.tile_pool(name="p_row", bufs=2))
        p_rec = ctx.enter_context(tc.tile_pool(name="p_rec", bufs=4))
        p_lg = ctx.enter_context(tc.tile_pool(name="p_lg", bufs=1))
        p_mu = ctx.enter_context(tc.tile_pool(name="p_mu", bufs=1))
        p_o32 = ctx.enter_context(tc.tile_pool(name="p_o32", bufs=1))
        ps_mm = ctx.enter_context(tc.tile_pool(name="ps_mm", bufs=2, space="PSUM"))
        ps_sc = ctx.enter_context(tc.tile_pool(name="ps_sc", bufs=2, space="PSUM"))
        ps_sm = ctx.enter_context(tc.tile_pool(name="ps_sm", bufs=2, space="PSUM"))
        ps_rc = ps_sm
        ps_cx = ctx.enter_context(tc.tile_pool(name="ps_cx", bufs=1, space="PSUM"))
        ps_bc = ctx.enter_context(tc.tile_pool(name="ps_bc", bufs=1, space="PSUM"))
        ps_st = ps_bc

        # ---- constants ----
        ones_col = consts.tile([128, 1], f16)       # 1/D: LN stats lhsT
        nc.vector.memset(ones_col, inv_d)
        ones_row = consts.tile([1, 128], f16)       # K=1 broadcast lhsT
        nc.vector.memset(ones_row, 1.0)
        sel2 = consts.tile([128, 2], f16)           # slot-sum lhsT
        nc.vector.memset(sel2, 0.0)
        nc.vector.memset(sel2[0:64, 0:1], 1.0)
        nc.vector.memset(sel2[64:128, 1:2], 1.0)
        selT = consts.tile([2, 128], f16)           # slot-broadcast lhsT
        nc.sync.dma_start(selT[:], selT_d.ap())
        eps_t = consts.tile([1, 1], f32)
        nc.vector.memset(eps_t, EPS)
        id64 = consts.tile([64, 64], f16)
        nc.sync.dma_start(id64[:], id64_d.ap())
        logbT = consts.tile([64, SC_W], f16)
        nc.sync.dma_start(logbT[:], logbT4.ap())

        per_layer = {}
        for l in range(L):
            d = {}
            for name, dram, w in [
                ("bq", bq_t, KD), ("bk", bk_t, KD), ("b1", b1_t, KF),
                ("c1", c1_t, KD), ("g1", g1_t, KD),
                ("c2", c2_t, KD), ("g2", g2_t, KD),
            ]:
                t = consts.tile([128, w], f32, tag=f"{name}_{l}")
                nc.sync.dma_start(t[:], dram.ap()[l])
                d[name] = t
            per_layer[l] = d
        go_sb = consts.tile([128, KD], f32)
        nc.sync.dma_start(go_sb[:], go_t.ap())
        boc_sb = consts.tile([128, KD], f32)
        nc.sync.dma_start(boc_sb[:], boc_t.ap())

        xreg = {}   # chunk -> current input tile (updated by LN2 / pass load)
        zreg = {}   # chunk -> LN1 output tile

        # ---------------- emission helpers ----------------

        def make_ln(s_get):
            """LayerNorm over [128,KD,TC] feature-major; two closures."""
            st = {}

            def q_stats():
                s_sb = s_get()
                sqs = []
                for m in range(KD):
                    sq = p_sq.tile([128, TC], f16, tag="sq")
                    nc.vector.tensor_tensor(sq[:], s_sb[:, m, :],
                                            s_sb[:, m, :], Alu.mult)
                    sqs.append(sq)
                stp = ps_st.tile([128, TC], f32, tag="bc", name="stp")
                for m in range(KD):
                    nc.tensor.matmul(stp[0:1, :], ones_col[:], s_sb[:, m, :],
                                     start=(m == 0), stop=(m == KD - 1),
                                     skip_group_check=True)
                for m in range(KD):
                    nc.tensor.matmul(stp[32:33, :], ones_col[:], sqs[m][:],
                                     start=(m == 0), stop=(m == KD - 1),
                                     tile_position=(0, 32),
                                     skip_group_check=True)
                mur = p_row.tile([1, TC], f16, tag="mur", bufs=1)
                nc.scalar.activation(mur[:], stp[0:1, :], Act.Copy)
                var = p_row.tile([1, TC], f32, tag="var", bufs=1)
                nc.vector.tensor_tensor(var[:], stp[0:1, :], mur[:],
                                        Alu.mult)
                nc.vector.tensor_tensor(var[:], stp[32:33, :], var[:],
                                        Alu.subtract)
                lg = p_row.tile([1, TC], f32, tag="lg", bufs=1)
                nc.scalar.activation(lg[:], var[:], Act.Ln, bias=eps_t[:])
                rsr = p_row.tile([1, TC], f16, tag="rsr", bufs=1)
                with nc.allow_low_precision(reason="fp16 rstd broadcast"):
                    nc.scalar.activation(rsr[:], lg[:], Act.Exp, scale=-0.5)
                st["mur"], st["rsr"] = mur, rsr

            def q_norm(out_sb=None, out_ap=None, gout=None, bout=None):
                s_sb = s_get()
                bcp = ps_bc.tile([128, TC], f32, tag="bc")
                nc.tensor.matmul(bcp[:], ones_row[:], st["mur"][:],
                                 start=True, stop=True)
                mub = p_mu.tile([128, TC], f16, tag="mub")
                nc.scalar.activation(mub[:], bcp[:], Act.Copy)
                bcp2 = ps_bc.tile([128, TC], f32, tag="bc")
                nc.tensor.matmul(bcp2[:], ones_row[:], st["rsr"][:],
                                 start=True, stop=True)
                rsb = p_mu.tile([128, TC], f16, tag="rsb")
                nc.scalar.activation(rsb[:], bcp2[:], Act.Copy)
                for m in range(KD):
                    t0 = p_t.tile([128, TC], f16, tag="t")
                    nc.vector.scalar_tensor_tensor(
                        t0[:], s_sb[:, m, :], 1.0, mub[:], Alu.mult,
                        Alu.subtract)
                    if out_ap is None:
                        nc.vector.tensor_tensor(out_sb[:, m, :], t0[:],
                                                rsb[:], Alu.mult)
                    else:
                        z32 = p_o32.tile([128, TC], f32, tag="o32")
                        nc.vector.tensor_tensor(z32[:], t0[:], rsb[:],
                                                Alu.mult)
                        nc.vector.tensor_scalar(
                            z32[:], z32[:], gout[:, m:m + 1],
                            bout[:, m:m + 1], Alu.mult, Alu.add)
                        nc.sync.dma_start(out_ap[:, m, :], z32[:])

            return q_stats, q_norm

        def attn_unit(l, c):
            """(stage_quanta, tail_quanta) for attention of chunk c."""
            cl = per_layer[l]
            state = {}

            def proj_half(wkey, okey, bias, ms):
                w_sl = state[wkey]
                out_sb = state[okey]
                xc = state["xc"]
                for m in ms:
                    ps = ps_mm.tile([128, TC], f32, tag="mm")
                    for k in range(KD):
                        nc.tensor.matmul(ps[:],
                                         w_sl[:, k, m * 128:(m + 1) * 128],
                                         xc[:, k, :],
                                         start=(k == 0), stop=(k == KD - 1))
                    nc.scalar.activation(out_sb[:, m, :], ps[:], Act.Identity,
                                         bias=bias[:, m:m + 1], scale=1.0)

            def s_q0():
                state["xc"] = xreg[c]
                wq = p_w.tile([128, KD, D], f16, tag="w")
                nc.sync.dma_start(
                    wq[:], Wq_s.ap()[l].rearrange("k p f -> p k f"))
                state["wq"] = wq
                state["q16"] = p_q.tile([128, KD, TC], f16, tag="q", name="q16")
                proj_half("wq", "q16", cl["bq"], range(0, 3))

            def s_q1():
                proj_half("wq", "q16", cl["bq"], range(3, KD))
                wk = p_w.tile([128, KD, D], f16, tag="w")
                nc.sync.dma_start(
                    wk[:], Wk_s.ap()[l].rearrange("k p f -> p k f"))
                state["wk"] = wk
                state["k16"] = p_k.tile([128, KD, TC], f16, tag="k", name="k16")

            def s_k0():
                proj_half("wk", "k16", cl["bk"], range(0, 3))

            def s_k1():
                proj_half("wk", "k16", cl["bk"], range(3, KD))
                wv = p_w.tile([128, KD, D], f16, tag="w")
                nc.sync.dma_start(
                    wv[:], Wv_s.ap()[l].rearrange("k p f -> p k f"))
                state["wv"] = wv
                state["v16"] = p_v.tile([128, BC // 2, D], f16, tag="v", name="v16")

            def v_pairs(prs):
                wv, v16, xc = state["wv"], state["v16"], state["xc"]
                for pr in prs:
                    for half in range(2):
                        ps = ps_mm.tile([128, TC], f32, tag="mm")
                        for k in range(KD):
                            nc.tensor.matmul(
                                ps[:, 0:384],
                                xc[:, k, pr * 128:(pr + 1) * 128],
                                wv[:, k, half * 384:(half + 1) * 384],
                                start=(k == 0), stop=(k == KD - 1))
                        nc.scalar.activation(
                            v16[:, pr, half * 384:(half + 1) * 384],
                            ps[:, 0:384], Act.Copy)

            def s_v0():
                state["vlo"] = p_vlo.tile([64, BC // 2, D], f16, tag="vlo",
                                          name="vlo")
                v_pairs(range(0, 2))
                for pr in range(0, 2):
                    nc.sync.dma_start(state["vlo"][0:64, pr, :],
                                      state["v16"][64:128, pr, :])

            def s_v1():
                v_pairs(range(2, BC // 2))
                for pr in range(2, BC // 2):
                    nc.sync.dma_start(state["vlo"][0:64, pr, :],
                                      state["v16"][64:128, pr, :])
                wo = p_w.tile([128, KD, D], f16, tag="w")
                nc.sync.dma_start(
                    wo[:], Wo_s.ap()[l].rearrange("k p f -> p k f"))
                state["wo"] = wo

            def scores_pair(hs):
                q16, k16 = state["q16"], state["k16"]
                exs = state.setdefault("ex", {})
                for h in hs:
                    scp = ps_sc.tile([128, SC_W], f32, tag="sc")
                    for s_ in range(2):
                        nc.tensor.matmul(
                            scp[64 * s_:64 * s_ + 64, :], id64[:], logbT[:],
                            start=True, stop=False,
                            tile_position=(0, 64 * s_), skip_group_check=True)
                    for b in range(BC):
                        s_, j = b % 2, b // 2
                        nc.tensor.matmul(
                            scp[64 * s_:64 * s_ + 64, 64 * j:64 * j + 64],
                            k16[:, h, b * 64:(b + 1) * 64],
                            q16[:, h, b * 64:(b + 1) * 64],
                            start=False, stop=(b == BC - 1),
                            skip_group_check=True)
                    ex = p_ex.tile([128, SC_W], f16, tag="ex")
                    nc.scalar.activation(ex[:], scp[:], Act.Exp)
                    exs[h] = ex

            stage = [s_q0, s_q1, s_k0, s_k1, s_v0, s_v1,
                     lambda: scores_pair((0, 1)),
                     lambda: scores_pair((2, 3)),
                     lambda: scores_pair((4, 5))]
            if dbg < 1:
                stage = stage[:6]

            def t_sums():
                recs = state.setdefault("rec", {})
                for h in range(H):
                    smp = ps_sm.tile([128, SC_W], f32, tag="smrc",
                                     name="smp")
                    nc.tensor.matmul(smp[0:2, :], sel2[:], state["ex"][h][:],
                                     start=True, stop=True)
                    lg = p_lg.tile([2, SC_W], f32, tag="lgs")
                    nc.scalar.activation(lg[:], smp[0:2, :], Act.Ln)
                    rec = p_rec.tile([2, SC_W], f16, tag="rec")
                    with nc.allow_low_precision(reason="softmax recip"):
                        nc.scalar.activation(rec[:], lg[:], Act.Exp,
                                             scale=-1.0)
                    recs[h] = rec

            def t_p16():
                p16s = state.setdefault("p16", {})
                for h in range(H):
                    rcp = ps_rc.tile([128, SC_W], f32, tag="smrc",
                                     name="rcp")
                    nc.tensor.matmul(rcp[:], selT[:], state["rec"][h][:],
                                     start=True, stop=True)
                    p16 = p_p.tile([128, SC_W], f16, tag="p")
                    nc.vector.tensor_tensor(p16[:], state["ex"][h][:],
                                            rcp[:], Alu.mult)
                    plo = p_plo.tile([64, SC_W], f16, tag="plo", name="plo")
                    nc.sync.dma_start(plo[0:64, :], p16[64:128, :])
                    p16s[h] = (p16, plo)

            def ctx_heads(hs):
                v16 = state["v16"]
                if "ctx16" not in state:
                    state["ctx16"] = p_ctx.tile([128, KD, TC], f16, tag="ctx", name="ctx16")
                ctx16 = state["ctx16"]
                vlo = state["vlo"]
                for h in hs:
                    p16, plo = state["p16"][h]
                    cxp = ps_cx.tile([128, TC], f32, tag="cx")
                    for b in range(BC):
                        s_, j = b % 2, b // 2
                        vt = v16 if s_ == 0 else vlo
                        pt = p16 if s_ == 0 else plo
                        nc.tensor.matmul(
                            cxp[:, b * 64:(b + 1) * 64],
                            vt[0:64, j, h * 128:(h + 1) * 128],
                            pt[0:64, 64 * j:64 * j + 64],
                            start=True, stop=True, skip_group_check=True)
                    nc.vector.tensor_copy(ctx16[:, h, :], cxp[:])

            def t_wo():
                wo, ctx16, xc = state["wo"], state["ctx16"], state["xc"]
                s1 = p_s.tile([128, KD, TC], f16, tag="s")
                state["s1"] = s1
                for m in range(KD):
                    ps = ps_mm.tile([128, TC], f32, tag="mm")
                    for k in range(KD):
                        nc.tensor.matmul(ps[:],
                                         wo[:, k, m * 128:(m + 1) * 128],
                                         ctx16[:, k, :],
                                         start=(k == 0), stop=(k == KD - 1))
                    t0 = p_t.tile([128, TC], f16, tag="t")
                    nc.vector.tensor_scalar(t0[:], xc[:, m, :],
                                            cl["g1"][:, m:m + 1],
                                            cl["c1"][:, m:m + 1],
                                            Alu.mult, Alu.add)
                    nc.vector.tensor_tensor(s1[:, m, :], t0[:], ps[:],
                                            Alu.add)

            q_stats, q_norm = make_ln(lambda: state["s1"])

            def t_norm():
                zc = p_x.tile([128, KD, TC], f16, tag=f"x{c}")
                q_norm(zc)
                zreg[c] = zc

            tail = [t_sums, t_p16,
                    lambda: ctx_heads(range(0, 3)),
                    lambda: ctx_heads(range(3, H)),
                    t_wo, q_stats, t_norm]
            tail = tail[:max(0, dbg - 1)]
            return stage, tail

        def ffn_unit(l, c, w1t, w2t, last, tok0):
            cl = per_layer[l]
            state = {}

            def w1_block(ms, hkey, moff):
                if hkey not in state:
                    state[hkey] = p_h.tile([128, KF2, TC], f16, tag=hkey, name=hkey)
                    if "zc" not in state:
                        state["zc"] = zreg[c]
                htile = state[hkey]
                zc = state["zc"]
                for m in ms:
                    ps = ps_mm.tile([128, TC], f32, tag="mm")
                    for k in range(KD):
                        nc.tensor.matmul(ps[:],
                                         w1t[:, k, m * 128:(m + 1) * 128],
                                         zc[:, k, :],
                                         start=(k == 0), stop=(k == KD - 1))
                    nc.scalar.activation(htile[:, m - moff, :], ps[:],
                                         Act.Relu,
                                         bias=cl["b1"][:, m:m + 1],
                                         scale=1.0)

            def w2_block(ms):
                if "s2" not in state:
                    state["s2"] = p_s.tile([128, KD, TC], f16, tag="s2", name="s2")
                s2, zc = state["s2"], state["zc"]
                ha, hb = state["ha"], state["hb"]
                for m in ms:
                    ps = ps_mm.tile([128, TC], f32, tag="mm")
                    for k in range(KF):
                        ht = ha if k < KF2 else hb
                        nc.tensor.matmul(ps[:],
                                         w2t[:, k, m * 128:(m + 1) * 128],
                                         ht[:, k % KF2, :],
                                         start=(k == 0), stop=(k == KF - 1))
                    t0 = p_t.tile([128, TC], f16, tag="t")
                    nc.vector.tensor_scalar(t0[:], zc[:, m, :],
                                            cl["g2"][:, m:m + 1],
                                            cl["c2"][:, m:m + 1],
                                            Alu.mult, Alu.add)
                    nc.vector.tensor_tensor(s2[:, m, :], t0[:], ps[:],
                                            Alu.add)

            stage = [lambda: w1_block(range(0, 6), "ha", 0),
                     lambda: w1_block(range(6, KF2), "ha", 0),
                     lambda: w1_block(range(KF2, 18), "hb", KF2),
                     lambda: w1_block(range(18, KF), "hb", KF2),
                     lambda: w2_block(range(0, 2)),
                     lambda: w2_block(range(2, 4)),
                     lambda: w2_block(range(4, KD))]

            q_stats, q_norm = make_ln(lambda: state["s2"])

            def g_norm():
                if last:
                    q_norm(out_ap=y.ap()[:, :, tok0:tok0 + TC],
                           gout=go_sb, bout=boc_sb)
                else:
                    xn = p_x.tile([128, KD, TC], f16, tag=f"x{c}")
                    q_norm(xn)
                    xreg[c] = xn

            tail = [q_stats, g_norm]
            return stage, tail

        def interleave(stage, prev_tail, mode_bit=3):
            if not (pipelined and (pipe_mode & mode_bit)):
                for fn in prev_tail:
                    fn()
                for fn in stage:
                    fn()
                return
            n = max(len(stage), len(prev_tail))
            for i in range(n):
                if i < len(prev_tail):
                    prev_tail[i]()
                if i < len(stage):
                    stage[i]()

        # ---------------- main program ----------------
        prev_tail = []
        for p in range(npass):
            for c in range(nchunk):
                tok0 = (p * nchunk + c) * TC
                xc = p_x.tile([128, KD, TC], f16, tag=f"x{c}")
                nc.sync.dma_start(
                    xc[:],
                    xw.ap()[:, :, tok0:tok0 + TC].rearrange("o p t -> p o t"))
                xreg[c] = xc
            for l in range(L):
                w1t = p_w1.tile([128, KD, FF], f16, tag="w1")
                nc.sync.dma_start(
                    w1t[:], W1_s.ap()[l].rearrange("k p f -> p k f"))
                w2t = p_w2.tile([128, KF, D], f16, tag="w2")
                nc.sync.dma_start(
                    w2t[:], W2_s.ap()[l].rearrange("k p f -> p k f"))
                for c in range(nchunk):
                    stage, tail = attn_unit(l, c)
                    interleave(stage, prev_tail, 1)
                    prev_tail = tail
                for c in range(nchunk):
                    if dbg < 9:
                        break
                    tok0 = (p * nchunk + c) * TC
                    stage, tail = ffn_unit(l, c, w1t, w2t,
                                           last=(l == L - 1), tok0=tok0)
                    interleave(stage, prev_tail, 2)
                    prev_tail = tail
        for fn in prev_tail:
            fn()

    nc.finalize()
    return nc


def _host_prep(inputs):
    f16 = np.float16
    f32 = np.float32
    scale = 1.0 / math.sqrt(HD)

    def arr(k):
        return np.asarray(inputs[k], dtype=np.float64)

    Wq, Wk, Wv, Wo = arr("Wq"), arr("Wk"), arr("Wv"), arr("Wo")
    W1, W2 = arr("W1"), arr("W2")
    bq, bk, bv, bo = arr("bq"), arr("bk"), arr("bv"), arr("bo")
    b1, b2 = arr("b1"), arr("b2")
    l1s, l1b = arr("ln1_s"), arr("ln1_b")
    l2s, l2b = arr("ln2_s"), arr("ln2_b")

    Wq_e = np.empty_like(Wq)
    Wk_e = np.empty_like(Wk)
    Wv_e = np.empty_like(Wv)
    W1_e = np.empty_like(W1)
    bq_e = np.empty_like(bq)
    bk_e = np.empty_like(bk)
    b1_e = np.empty_like(b1)
    c1 = np.empty_like(bo)
    g1 = np.empty_like(bo)
    c2 = np.empty_like(b2)
    g2 = np.empty_like(b2)
    for l in range(L):
        gp = l2s[l - 1] if l > 0 else np.ones(D)
        bp = l2b[l - 1] if l > 0 else np.zeros(D)
        Wq_e[l] = Wq[l] * gp[:, None] * scale
        bq_e[l] = (bq[l] + bp @ Wq[l]) * scale
        Wk_e[l] = Wk[l] * gp[:, None]
        bk_e[l] = bk[l] + bp @ Wk[l]
        Wv_e[l] = Wv[l] * gp[:, None]
        c1[l] = bp + bo[l] + (bv[l] + bp @ Wv[l]) @ Wo[l]
        g1[l] = gp
        W1_e[l] = W1[l] * l1s[l][:, None]
        b1_e[l] = b1[l] + l1b[l] @ W1[l]
        c2[l] = l1b[l] + b2[l]
        g2[l] = l1s[l]

    def slabs(w, nk):
        return np.ascontiguousarray(
            w.reshape(L, nk, 128, w.shape[-1])).astype(f16)

    def cols(b, nk):  # [L, feat] -> [L, 128, nk]
        return np.ascontiguousarray(
            b.astype(f32).reshape(L, nk, 128).transpose(0, 2, 1))

    bias = np.asarray(inputs["sp_table"], np.float64)[
        np.asarray(inputs["sp_matrix"])]
    logbT = np.ascontiguousarray(bias.T)          # [k, q]
    logbT4 = np.tile(logbT, (1, BC // 2)).astype(f16)

    prep = {
        "Wq_s": slabs(Wq_e, KD), "Wk_s": slabs(Wk_e, KD),
        "Wv_s": slabs(Wv_e, KD), "Wo_s": slabs(Wo, KD),
        "W1_s": slabs(W1_e, KD), "W2_s": slabs(W2, KF),
        "bq_t": cols(bq_e, KD), "bk_t": cols(bk_e, KD),
        "b1_t": cols(b1_e, KF),
        "c1_t": cols(c1, KD), "g1_t": cols(g1, KD),
        "c2_t": cols(c2, KD), "g2_t": cols(g2, KD),
        "go_t": np.ascontiguousarray(
            l2s[2].astype(f32).reshape(KD, 128).T),
        "boc_t": np.ascontiguousarray(
            l2b[2].astype(f32).reshape(KD, 128).T),
        "logbT4": np.ascontiguousarray(logbT4),
        "id64_d": np.eye(64, dtype=f16),
        "selT_d": np.kron(np.eye(2, dtype=f16), np.ones((1, 64), f16)),
    }
    prep = {k: np.ascontiguousarray(v) for k, v in prep.items()}

    x = np.asarray(inputs["x"])
    x16 = x.astype(f16).reshape(NCORES, TOK, KD, 128)
    xw = np.ascontiguousarray(x16.transpose(0, 2, 3, 1))
    return prep, xw


def kernel(**inputs) -> np.ndarray:
    from concourse import bass_utils

    npass = int(inputs.pop("_npass", NPASS))
    nchunk = int(inputs.pop("_nchunk", NCH))
    dbg = int(inputs.pop("_dbg", 9))
    trace = bool(inputs.pop("_trace", False))

    key = (npass, nchunk, dbg)
    if key not in _CACHED_NC:
        _CACHED_NC[key] = _build_nc(npass, nchunk, dbg)
    nc = _CACHED_NC[key]

    prep, xw = _host_prep(inputs)
    in_maps = [dict(prep, xw=np.ascontiguousarray(xw[c]))
               for c in range(NCORES)]

    res = bass_utils.run_bass_kernel_spmd(
        nc, in_maps, core_ids=list(range(NCORES)), trace=trace)
    kernel.last_result = res

    ntok = npass * nchunk * TC
    out = np.zeros((B, S, D), dtype=np.float32)
    for core in range(NCORES):
        yc = res.results[core]["y"][:, :, :ntok]          # [128, KD, ntok]
        oc = yc.reshape(128, KD, ntok // S, S).transpose(2, 3, 1, 0)
        out[core * BS: core * BS + ntok // S] = oc.reshape(ntok // S, S, D)
    return out


kernel.last_result = None
